# revision 17
# baseline (speedup 1.0000x reference)
"""BandSplitEncoder Trainium2 kernel.

x[B,T,2048] is split into 62 bands (widths 4..256); each band is
RMS-normalized (L2 norm * sqrt(d) * gamma) and passed through its own
Linear[d -> 512]; outputs stack to [B,T,62,512].

Sharding: hybrid 4x2 over 8 cores -- 4 token groups (512 tokens each)
x 2 band groups (31 bands / 1024 features each) with identical width
multisets, so one SPMD program serves all cores.  Per core the output
is 512*31*512 fp16 = 16.25MB; at the ~358GB/s per-core HBM limit the
output stream is a ~46us floor, and the PSUM->SBUF copy engines (ACT
~1.05us, DVE ~1.18us per 2-bank pair) are a parallel ~46us floor.

Per-core dataflow (matmul operands bf16, f32 PSUM, f16 out):
  1. xt (packed transposed activations, 13 zero-padded 128-row chunks
     holding up to 4 bands in 32-row PE slots) streams in on the sync
     HWDGE ring; wg (packed folded weights) and the indicator matrices
     on the scalar ring, both issuing concurrently.  8 junk warm-up
     matmuls run during the input window so the PE HAM clock gate is
     at 2.4GHz when real matmuls start; a dummy reciprocal_approx_fast
     preloads its custom-DVE ucode table.
  2. Norms come from the packed image itself (no separate natural-
     layout copy of x): in 5 pipelined chunk-groups ([0],[1],[2,3],
     [4-7],[8-12]), xsq = xt*xt (DVE, batched per group), one
     indicator matmul per chunk (stationary IND [128,nb]) reduces band
     rows -> ssqT[band, tok] in PSUM, then reciprocal_approx_fast
     (DVE, ~18 bits -- plenty for a bf16 scale) + sqrt (ACT) give
     invT[band, tok] bf16.  The d256 band's two chunks accumulate into
     one ssqT slot.
  3. The inv-norm folds into the activations BEFORE the matmul:
     smap = indT.T @ invT broadcasts each band's inv-norm row over its
     feature rows (one small matmul per chunk), xts = xt * smap (DVE).
     This makes the output-side copies PURE so they can batch 2 PSUM
     banks per instruction; a per-(band,token) scale on the copy would
     pin them to one bank and ~25% more engine time.
  4. Mains in 5 out-groups (4/8/8/6/5 bands) aligned with norm-group
     readiness, prologue k+1 emitted between main groups so engine
     FIFOs never serialize phases: per (band, token-tile) matmul
     psum = xts_strip.T @ wg_band with PE row tiling (tile_position)
     packing up to 4 bands per 128-row chunk; pair-of-banks pure
     copies alternate ACT:DVE 4:3; two-token-tile DMAs (1..2MB)
     stream on the sync ring from ~20us to the end.
PSUM: 3 pair slots (6 banks) for the mains ring + 1 ssq + 1 smap bank.
gamma*sqrt(d) is folded into W on the host; b is added on the host (it
broadcasts over tokens).  Measured ~76us vs the 71.4-85.8us (high
run-to-run variance) of the f16 data-parallel predecessor.
"""

import numpy as np
import ml_dtypes

import concourse.bacc as bacc
import concourse.tile as tile
from concourse import mybir
from concourse.bass_utils import run_bass_kernel_spmd

# ---------------------------------------------------------------- problem dims
DIM_INPUTS = (4,) * 24 + (8,) * 12 + (24,) * 8 + (48,) * 8 + (96,) * 8 + (256,) * 2
N_BANDS = len(DIM_INPUTS)  # 62
F_TOTAL = sum(DIM_INPUTS)  # 2048
DIM = 512
B, T = 4, 512
BT = B * T  # 2048 tokens
N_CORES = 8
N_TG = 4  # token groups
N_BG = 2  # band groups
TOK = BT // N_TG  # 512 tokens per core
N_TILES = TOK // 128  # 4 token tiles per core

OFFSETS = []
_off = 0
for _d in DIM_INPUTS:
    OFFSETS.append(_off)
    _off += _d

# global band ids per width class
_D4 = list(range(0, 24))
_D8 = list(range(24, 36))
_D24 = list(range(36, 44))
_D48 = list(range(44, 52))
_D96 = list(range(52, 60))
_D256 = [60, 61]

# band groups: same width multiset so one SPMD program serves both
GROUP_BANDS = []
for g in range(N_BG):
    h = lambda lst: lst[len(lst) // 2 * g : len(lst) // 2 * (g + 1)]
    d4, d8, d24, d48, d96, d256 = h(_D4), h(_D8), h(_D24), h(_D48), h(_D96), h(_D256)
    # local chunk geometry (identical for both groups):
    #  C0-C2: 4x d4 each    C3: 4x d8          C4: d8,d8,d24,d24
    #  C5: d96,d24  C6: d96,d24  C7: d96  C8: d96
    #  C9: d48,d48  C10: d48,d48  C11+C12: d256 (two segments)
    bands = []
    for i in range(12):  # d4 -> C0..C2
        bands.append((d4[i], [(i // 4, 32 * (i % 4), 4)]))
    for i in range(4):  # first 4 d8 -> C3
        bands.append((d8[i], [(3, 32 * i, 8)]))
    bands.append((d8[4], [(4, 0, 8)]))
    bands.append((d8[5], [(4, 32, 8)]))
    bands.append((d24[0], [(4, 64, 24)]))
    bands.append((d24[1], [(4, 96, 24)]))
    bands.append((d96[0], [(5, 0, 96)]))
    bands.append((d24[2], [(5, 96, 24)]))
    bands.append((d96[1], [(6, 0, 96)]))
    bands.append((d24[3], [(6, 96, 24)]))
    bands.append((d96[2], [(7, 0, 96)]))
    bands.append((d96[3], [(8, 0, 96)]))
    bands.append((d48[0], [(9, 0, 48)]))
    bands.append((d48[1], [(9, 64, 48)]))
    bands.append((d48[2], [(10, 0, 48)]))
    bands.append((d48[3], [(10, 64, 48)]))
    bands.append((d256[0], [(11, 0, 128), (12, 0, 128)]))
    GROUP_BANDS.append(bands)

NB_LOCAL = len(GROUP_BANDS[0])  # 31
N_CHUNKS = 13
F_PACK = N_CHUNKS * 128  # 1664

# per-chunk band lists (geometry identical across groups): (local_band, slot, n)
CHUNK_BANDS = [[] for _ in range(N_CHUNKS)]
for lb, (_, segs) in enumerate(GROUP_BANDS[0]):
    for c, slot, n in segs:
        CHUNK_BANDS[c].append((lb, slot, n))
NB_CHUNK = [len(CHUNK_BANDS[c]) for c in range(N_CHUNKS)]

# norm chunk-groups (pipelined; first two small so the out stream starts
# early); chunk 12 accumulates into chunk 11's slot (same d256 band)
NORM_GROUPS = [[0], [1], [2, 3], [4, 5, 6, 7], [8, 9, 10, 11, 12]]
CHUNK_NORM = {}  # chunk -> (group, col_slot, accumulate_flag)
for g, cs in enumerate(NORM_GROUPS):
    pos = 0
    for c in cs:
        if c == 12:
            CHUNK_NORM[c] = (g, CHUNK_NORM[11][1], True)
        else:
            CHUNK_NORM[c] = (g, 32 * pos, False)
            pos += 1

# packed row -> source feature row (or -1), per band group
ROW_MAPS = []
for g in range(N_BG):
    rm = np.full((F_PACK,), -1, dtype=np.int64)
    for gb, segs in GROUP_BANDS[g]:
        src = OFFSETS[gb]
        for c, slot, n in segs:
            rm[c * 128 + slot : c * 128 + slot + n] = np.arange(src, src + n)
            src += n
    ROW_MAPS.append(rm)

# out-groups sized so group k's chunks are ready after norm-groups
# emitted before it: chunks {0} {1,2} {3,4} {5-8} {9-12}
OUT_GROUPS = [list(range(0, 4)), list(range(4, 12)), list(range(12, 20)),
              list(range(20, 26)), list(range(26, 31))]
N_T2 = N_TILES // 2  # two-token-tile output DMA batches

_CACHE = {}


def _build_program():
    nc = bacc.Bacc("TRN2", target_bir_lowering=False, debug=False, num_devices=N_CORES)
    f32 = mybir.dt.float32
    f16 = mybir.dt.float16
    bf16 = mybir.dt.bfloat16
    AF = mybir.ActivationFunctionType
    OP = mybir.AluOpType

    xt_ap = nc.dram_tensor("xt", [128, N_CHUNKS * TOK], bf16, kind="ExternalInput").ap()
    wg_ap = nc.dram_tensor("wg", [128, N_CHUNKS * DIM], bf16, kind="ExternalInput").ap()
    ind_ap = nc.dram_tensor("ind", [128, N_CHUNKS * 4], bf16, kind="ExternalInput").ap()
    indt_ap = nc.dram_tensor("indt", [128, N_CHUNKS * 128], bf16, kind="ExternalInput").ap()
    out_ap = nc.dram_tensor("out", [TOK, NB_LOCAL * DIM], f16, kind="ExternalOutput").ap()

    with tile.TileContext(nc) as tc:
        with (
            tc.tile_pool(name="const", bufs=1) as const_pool,
            tc.tile_pool(name="xsq", bufs=2) as xsq_pool,
            tc.tile_pool(name="rsq", bufs=2) as rsq_pool,
            tc.tile_pool(name="invt", bufs=2) as invt_pool,
            tc.tile_pool(name="outb", bufs=4) as out_pool,
            tc.tile_pool(name="ps", bufs=3, space="PSUM") as psum_pool,
        ):
            # ---- PE warm-up on junk data while inputs stream in (~3.5us of
            # matmuls flips the HAM clock gate to 2.4GHz); dummy approx-recip
            # preloads its custom-DVE ucode table outside the critical path
            WUP = const_pool.tile([128, 640], bf16, name="wup")
            nc.gpsimd.memset(WUP[:], 0)
            WUPF = const_pool.tile([128, 8], f32, name="wupf")
            nc.gpsimd.memset(WUPF[:], 1)
            WUPR = const_pool.tile([128, 8], f32, name="wupr")
            nc.vector.reciprocal_approx_fast(out=WUPR[:], in_=WUPF[:])
            wps = psum_pool.tile([128, DIM], f32, name="wps", tag="ssq", bufs=1, space="PSUM")
            for _ in range(8):
                nc.tensor.matmul(wps[:], WUP[:, 0:128], WUP[:, 128:640],
                                 start=True, stop=True)

            # ---- inputs: xt on the sync ring, weights/indicators on the
            # scalar ring -- both rings issue concurrently
            XT = const_pool.tile([128, N_CHUNKS * TOK], bf16, name="xt")
            WG = const_pool.tile([128, N_CHUNKS * DIM], bf16, name="wg")
            IND = const_pool.tile([128, N_CHUNKS * 4], bf16, name="ind")
            INDT = const_pool.tile([128, N_CHUNKS * 128], bf16, name="indt")
            nc.sync.dma_start(XT[:, 0 : TOK], xt_ap[:, 0 : TOK])
            nc.scalar.dma_start(IND[:], ind_ap[:])
            nc.scalar.dma_start(INDT[:], indt_ap[:])
            nc.sync.dma_start(XT[:, TOK : 2 * TOK], xt_ap[:, TOK : 2 * TOK])
            nc.sync.dma_start(WG[:, 0 : 2 * DIM], wg_ap[:, 0 : 2 * DIM])
            nc.sync.dma_start(XT[:, 2 * TOK : 4 * TOK], xt_ap[:, 2 * TOK : 4 * TOK])
            nc.sync.dma_start(XT[:, 4 * TOK : 8 * TOK], xt_ap[:, 4 * TOK : 8 * TOK])
            nc.sync.dma_start(WG[:, 2 * DIM : 4 * DIM], wg_ap[:, 2 * DIM : 4 * DIM])
            nc.sync.dma_start(XT[:, 8 * TOK :], xt_ap[:, 8 * TOK :])
            nc.sync.dma_start(WG[:, 4 * DIM : 8 * DIM], wg_ap[:, 4 * DIM : 8 * DIM])
            nc.sync.dma_start(WG[:, 8 * DIM :], wg_ap[:, 8 * DIM :])

            XTS = const_pool.tile([128, N_CHUNKS * TOK], bf16, name="xts")
            SMAP_OF = {}  # chunk -> psum tile

            def prologue(g):
                chunks = NORM_GROUPS[g]
                nch = len([c for c in chunks if c != 12])
                c0 = chunks[0]
                # batched squares: group 0 on DVE (latency-critical),
                # the rest on the otherwise-idle GPSIMD engine
                XSQ = xsq_pool.tile([128, len(chunks) * TOK], bf16,
                                    name=f"xsq{g}", tag="xsq")
                eng = nc.vector
                eng.tensor_tensor(
                    XSQ[:], XT[:, c0 * TOK : (c0 + len(chunks)) * TOK],
                    XT[:, c0 * TOK : (c0 + len(chunks)) * TOK], OP.mult,
                )
                SSQ = psum_pool.tile([128, TOK], f32, name=f"ssq{g}", tag="ssq",
                                     bufs=1, space="PSUM")
                for c in chunks:
                    _, slot, accum = CHUNK_NORM[c]
                    nb = NB_CHUNK[c]
                    nc.tensor.matmul(
                        SSQ[slot : slot + nb, :],
                        IND[:, c * 4 : c * 4 + nb],
                        XSQ[:, (c - c0) * TOK : (c - c0 + 1) * TOK],
                        start=not accum,
                        stop=(c != 11),
                        tile_position=(0, slot),
                    )
                RSQ = rsq_pool.tile([128, TOK], f32, name=f"rsq{g}", tag="rsq")
                nc.vector.reciprocal_approx_fast(out=RSQ[:], in_=SSQ[:])
                INVT = invt_pool.tile([128, TOK], bf16, name=f"invt{g}", tag="invt")
                nc.scalar.activation(INVT[:], RSQ[:], AF.Sqrt)

                # per-chunk smap broadcast matmul + xts multiply
                for c in chunks:
                    _, slot, _ = CHUNK_NORM[c]
                    nb = NB_CHUNK[c]
                    if c == 12:  # d256 second segment: same inv-norm rows
                        SM = SMAP_OF[11]
                    else:
                        SM = psum_pool.tile([128, TOK], f32, name=f"smap{c}",
                                            tag="smap", bufs=1, space="PSUM")
                        nc.tensor.matmul(
                            SM[:],
                            INDT[slot : slot + nb, c * 128 : (c + 1) * 128],
                            INVT[slot : slot + nb, :],
                            start=True, stop=True,
                            tile_position=(slot, 0),
                        )
                        SMAP_OF[c] = SM
                    nc.vector.tensor_tensor(
                        XTS[:, c * TOK : (c + 1) * TOK],
                        XT[:, c * TOK : (c + 1) * TOK], SM[:], OP.mult,
                    )

            copy_idx = 0
            # zipper: prologue ops for later norm-groups are emitted one at a
            # time between main pairs, so their ACT/DVE cost rides in the
            # copy-pipeline's slack instead of stalling it at group boundaries
            pending = []

            def pump():
                if pending:
                    pending.pop(0)()

            def make_thunks(g):
                chunks = NORM_GROUPS[g]
                c0 = chunks[0]
                st = {}

                def sq():
                    st["XSQ"] = xsq_pool.tile([128, len(chunks) * TOK], bf16,
                                              name=f"xsq{g}", tag="xsq")
                    nc.vector.tensor_tensor(
                        st["XSQ"][:], XT[:, c0 * TOK : (c0 + len(chunks)) * TOK],
                        XT[:, c0 * TOK : (c0 + len(chunks)) * TOK], OP.mult,
                    )

                def nmm():
                    st["SSQ"] = psum_pool.tile([128, TOK], f32, name=f"ssq{g}",
                                               tag="ssq", bufs=1, space="PSUM")
                    for c in chunks:
                        _, slot, accum = CHUNK_NORM[c]
                        nb = NB_CHUNK[c]
                        nc.tensor.matmul(
                            st["SSQ"][slot : slot + nb, :],
                            IND[:, c * 4 : c * 4 + nb],
                            st["XSQ"][:, (c - c0) * TOK : (c - c0 + 1) * TOK],
                            start=not accum,
                            stop=(c != 11),
                            tile_position=(0, slot),
                        )

                def rec():
                    st["RSQ"] = rsq_pool.tile([128, TOK], f32, name=f"rsq{g}",
                                              tag="rsq")
                    nc.vector.reciprocal_approx_fast(out=st["RSQ"][:],
                                                     in_=st["SSQ"][:])

                def sqr():
                    st["INVT"] = invt_pool.tile([128, TOK], bf16,
                                                name=f"invt{g}", tag="invt")
                    nc.scalar.activation(st["INVT"][:], st["RSQ"][:], AF.Sqrt)

                def make_st(c):
                    def f():
                        _, slot, _ = CHUNK_NORM[c]
                        nb = NB_CHUNK[c]
                        if c == 12:  # d256 second segment: same inv-norm rows
                            SM = SMAP_OF[11]
                        else:
                            SM = psum_pool.tile([128, TOK], f32, name=f"smap{c}",
                                                tag="smap", bufs=1, space="PSUM")
                            nc.tensor.matmul(
                                SM[:],
                                INDT[slot : slot + nb, c * 128 : (c + 1) * 128],
                                st["INVT"][slot : slot + nb, :],
                                start=True, stop=True,
                                tile_position=(slot, 0),
                            )
                            SMAP_OF[c] = SM
                        nc.vector.tensor_tensor(
                            XTS[:, c * TOK : (c + 1) * TOK],
                            XT[:, c * TOK : (c + 1) * TOK], SM[:], OP.mult,
                        )
                    return f

                return [sq, nmm, rec, sqr] + [make_st(c) for c in chunks]

            def mains_fine(gi):
                # single-token-tile DMAs: lowest latency to first byte
                nonlocal copy_idx
                group = OUT_GROUPS[gi]
                g0 = group[0]
                for t in range(N_TILES):
                    OUT = out_pool.tile([128, 8 * DIM], f16, name="outf", tag="outf")
                    j = 0
                    while j < len(group):
                        pair = group[j : j + 2]
                        ps = psum_pool.tile([128, len(pair) * DIM], f32,
                                            name="psp", tag="ps", space="PSUM")
                        for k, lb in enumerate(pair):
                            segs = GROUP_BANDS[0][lb][1]
                            for si, (c, slot, n) in enumerate(segs):
                                nc.tensor.matmul(
                                    ps[:, k * DIM : (k + 1) * DIM],
                                    XTS[slot : slot + n,
                                        c * TOK + t * 128 : c * TOK + (t + 1) * 128],
                                    WG[slot : slot + n, c * DIM : (c + 1) * DIM],
                                    start=(si == 0),
                                    stop=(si == len(segs) - 1),
                                    tile_position=(slot, 0),
                                )
                        dst = OUT[:, j * DIM : (j + len(pair)) * DIM]
                        if copy_idx % 7 in (0, 2, 4, 6):
                            nc.scalar.activation(dst, ps[:], AF.Copy)
                        else:
                            nc.vector.tensor_copy(dst, ps[:])
                        copy_idx += 1
                        j += len(pair)
                        pump()
                    nc.sync.dma_start(
                        out_ap[t * 128 : (t + 1) * 128,
                               g0 * DIM : (g0 + len(group)) * DIM],
                        OUT[:, 0 : len(group) * DIM],
                    )
                    pump()

            def mains(gi):
                nonlocal copy_idx
                group = OUT_GROUPS[gi]
                g0 = group[0]
                for t2 in range(N_T2):
                    OUT = out_pool.tile([128, 2 * 8 * DIM], f16, name="outt",
                                        tag="outt")
                    for i in range(2):
                        t = 2 * t2 + i
                        j = 0
                        while j < len(group):
                            pair = group[j : j + 2]
                            ps = psum_pool.tile([128, len(pair) * DIM], f32,
                                                name="psp", tag="ps", space="PSUM")
                            for k, lb in enumerate(pair):
                                segs = GROUP_BANDS[0][lb][1]
                                for si, (c, slot, n) in enumerate(segs):
                                    nc.tensor.matmul(
                                        ps[:, k * DIM : (k + 1) * DIM],
                                        XTS[slot : slot + n,
                                            c * TOK + t * 128 : c * TOK + (t + 1) * 128],
                                        WG[slot : slot + n, c * DIM : (c + 1) * DIM],
                                        start=(si == 0),
                                        stop=(si == len(segs) - 1),
                                        tile_position=(slot, 0),
                                    )
                            dst = OUT[:, (i * 8 + j) * DIM : (i * 8 + j + len(pair)) * DIM]
                            if copy_idx % 7 in (0, 2, 4, 6):  # ACT:DVE = 4:3
                                nc.scalar.activation(dst, ps[:], AF.Copy)
                            else:
                                nc.vector.tensor_copy(dst, ps[:])
                            copy_idx += 1
                            j += len(pair)
                            pump()
                    nc.sync.dma_start(
                        out_ap[t2 * 256 : (t2 + 1) * 256,
                               g0 * DIM : (g0 + len(group)) * DIM]
                        .rearrange("(i p) n -> p i n", p=128),
                        OUT[:].rearrange("p (i n) -> p i n", i=2)[
                            :, :, 0 : len(group) * DIM],
                    )
                    pump()

            # group 0's prologue emits directly (shortest head chain); later
            # groups' prologues drip through the zipper during the mains.
            # t1/t2 hand-interleaved so the ssq ring (bufs=1) reuse order
            # matches emission order and dependent ops sit >=2 pops apart.
            prologue(0)
            prologue(1)
            mains(0)   # chunks {0}
            prologue(2)
            mains(1)   # chunks {1,2}
            prologue(3)
            mains(2)   # chunks {3,4}
            prologue(4)
            mains(3)   # chunks {5,6,7,8,9}
            mains(4)   # chunks {9,10,11,12}

    nc.compile()
    return nc


def _get_program():
    if "nc" not in _CACHE:
        _CACHE["nc"] = _build_program()
    return _CACHE["nc"]


def _pack_inputs(x, gamma, W):
    """Host-side packing: fold gamma*sqrt(d) into W, build per-band-group
    packed SBUF images and the indicator matrices."""
    bf = ml_dtypes.bfloat16
    xf = np.ascontiguousarray(np.asarray(x, dtype=np.float32).reshape(BT, F_TOTAL))
    gamma = np.asarray(gamma, dtype=np.float32)
    W = np.asarray(W, dtype=np.float32)

    scale = np.empty((F_TOTAL,), dtype=np.float32)
    for b_i, d in enumerate(DIM_INPUTS):
        scale[OFFSETS[b_i] : OFFSETS[b_i] + d] = np.float32(np.sqrt(d))
    wfold = (gamma * scale)[:, None] * W  # [2048, 512]

    # indicator images (geometry shared by both groups)
    ind = np.zeros((128, N_CHUNKS * 4), dtype=np.float32)
    indt = np.zeros((128, N_CHUNKS * 128), dtype=np.float32)
    for c in range(N_CHUNKS):
        for k, (lb, slot, n) in enumerate(CHUNK_BANDS[c]):
            ind[slot : slot + n, c * 4 + k] = 1.0
            _, nslot, _ = CHUNK_NORM[c]
            indt[nslot + k, c * 128 + slot : c * 128 + slot + n] = 1.0
    ind = ind.astype(bf)
    indt = indt.astype(bf)

    wgs = []
    for g in range(N_BG):
        rm = ROW_MAPS[g]
        valid = rm >= 0
        wp = np.zeros((F_PACK, DIM), dtype=np.float32)
        wp[valid] = wfold[rm[valid]]
        wp = np.ascontiguousarray(
            wp.astype(bf).reshape(N_CHUNKS, 128, DIM).transpose(1, 0, 2)
        ).reshape(128, N_CHUNKS * DIM)
        wgs.append(wp)

    in_maps = []
    for core in range(N_CORES):
        tg, bg = core // N_BG, core % N_BG
        rm = ROW_MAPS[bg]
        valid = rm >= 0
        shard = xf[tg * TOK : (tg + 1) * TOK]  # [512, 2048]
        xtp = np.zeros((F_PACK, TOK), dtype=np.float32)
        xtp[valid] = shard.T[rm[valid]]
        xtp = np.ascontiguousarray(
            xtp.astype(bf).reshape(N_CHUNKS, 128, TOK).transpose(1, 0, 2)
        ).reshape(128, N_CHUNKS * TOK)
        in_maps.append({"xt": xtp, "wg": wgs[bg], "ind": ind, "indt": indt})
    return in_maps


def _run(x, gamma, W, b, trace=False, trace_kwargs=None):
    nc = _get_program()
    b = np.asarray(b, dtype=np.float32)
    in_maps = _pack_inputs(x, gamma, W)

    kw = {}
    if trace:
        kw = {"trace": True, "trace_kwargs": trace_kwargs or {}}
    res = run_bass_kernel_spmd(nc, in_maps, core_ids=list(range(N_CORES)), **kw)

    out = np.empty((BT, N_BANDS, DIM), dtype=np.float32)
    for core in range(N_CORES):
        tg, bg = core // N_BG, core % N_BG
        r = res.results[core]["out"].astype(np.float32).reshape(TOK, NB_LOCAL, DIM)
        globs = [gb for gb, _ in GROUP_BANDS[bg]]
        out[tg * TOK : (tg + 1) * TOK, globs, :] = r
    out = out.reshape(B, T, N_BANDS, DIM)
    out += b[None, None, :, :]
    return out, res


def kernel(x, gamma, W, b):
    out, _ = _run(x, gamma, W, b)
    return out
